# revision 1
# baseline (speedup 1.0000x reference)
"""GraphAttentionLayer (GAT) Bass kernel for Trainium2, 8 NeuronCores.

Problem: B=8, N=2048, Fin=256, Fout=64
    Wh  = h @ W                                   [B, N, 64]
    e   = Wh@a1 + (Wh@a2)^T  (additive scores)    [B, N, N]
    att = where(adj>0, leaky_relu(e, 0.2), -9e15)
    A   = softmax(att, axis=1)   (column softmax!)
    out = elu(A @ Wh)

Sharding: batch-parallel, one graph per core (no communication).

Per-core dataflow (transposed layout, m = attended-over node on partitions,
n = output node along free axis; m-tiles of 128):
    mm1 (PE):   Wh_psum[m,0:64] = hT.T @ W ; cols 64:66 = Wh@[a2, 0.2*a2]
    leaky (ACT+DVE split by column band):
        ACT:  Prelu(Wh1_bcast + Wh2[m], alpha=0.2)        cols [0:C_ACT)
        DVE:  max(e, 0.2e) via tensor_scalar + STT        cols [C_ACT:2048)
    mask (SWDGE accum-DMA): att += adj8 (fp8e5 {0,-57344} cast to f32)
    exp (ACT): P = Exp(att), accum_out -> den[m]
    fold (DVE): whp = Wh[m,0:64] * (1/den[m])
    mm2 (PE):   out_T[o,n] += whp.T @ P   (accumulate over 16 m-tiles)
    elu tail:   out = relu(x) + min(exp(x)-1, 0)
Host: transposes h/adj per batch, encodes adj as fp8, transposes output back.

The attention math is exact vs the reference: softmax without max-subtraction
is algebraically identical (exp values stay well inside fp32 range), masked
entries produce exp(att - 57344) == 0 exactly, and the 1/den fold is applied
to the contraction operand Wh.
"""

import contextlib
import sys

import numpy as np

if "/opt/trn_rl_repo" not in sys.path:
    sys.path.append("/opt/trn_rl_repo")

import ml_dtypes
import concourse.bass as bass
import concourse.bacc as bacc
import concourse.mybir as mybir
import concourse.tile as tile
from concourse import bass_utils

B = 8
N = 2048
FIN = 256
FOUT = 64
NT = N // 128          # 16 m-tiles
ALPHA = 0.2
MASK8 = 57344.0        # max fp8e5m2 magnitude; exp(x - 57344) == 0 for our x
C_ACT = 896            # leaky-relu columns done on ACT; rest on DVE
DT = mybir.dt.float32
AF = mybir.ActivationFunctionType
ALU = mybir.AluOpType

_CACHE = {}


def build_program(reps: int = 1, loop_k: int = 0):
    """Build and compile the SPMD single-core program (identical on 8 cores).

    reps statically unrolls the main body; loop_k wraps it in a dynamic
    For_i loop instead (constant program size -- used for timing).
    """
    nc = bacc.Bacc(
        "TRN2",
        target_bir_lowering=False,
        debug=False,
        enable_asserts=False,
        num_devices=B,
    )
    hT_d = nc.dram_tensor("hT", [FIN, N], DT, kind="ExternalInput")
    W_d = nc.dram_tensor("W", [FIN, FOUT], DT, kind="ExternalInput")
    arow_d = nc.dram_tensor("arow", [1, 2 * FOUT], DT, kind="ExternalInput")
    adj8_d = nc.dram_tensor("adj8", [N, N], mybir.dt.float8e5, kind="ExternalInput")
    out_d = nc.dram_tensor("out", [FOUT, N], DT, kind="ExternalOutput")

    with tile.TileContext(nc) as tc:
        with (
            tc.tile_pool(name="const", bufs=1) as const,
            tc.tile_pool(name="psmall", bufs=3, space=bass.MemorySpace.PSUM) as psmall,
            tc.tile_pool(name="pbig", bufs=1, space=bass.MemorySpace.PSUM) as pbig,
            tc.tile_pool(name="watt", bufs=3) as watt,
            tc.tile_pool(name="wp", bufs=3) as wp,
            tc.tile_pool(name="wut", bufs=2) as wut,
            tc.tile_pool(name="wsm", bufs=4) as wsm,
            tc.tile_pool(name="wout", bufs=1) as wout,
        ):
            # ---- load inputs ----
            hT = [const.tile([128, N], DT, name=f"hT{i}", tag=f"hT{i}") for i in range(2)]
            Wsb = [const.tile([128, FOUT], DT, name=f"W{i}", tag=f"W{i}") for i in range(2)]
            arow = const.tile([1, 2 * FOUT], DT, name="arow", tag="arow")
            for i in range(2):
                nc.sync.dma_start(hT[i][:], hT_d.ap()[i * 128:(i + 1) * 128, :])
                nc.sync.dma_start(Wsb[i][:], W_d.ap()[i * 128:(i + 1) * 128, :])
            nc.sync.dma_start(arow[:], arow_d.ap())

            # ---- a broadcast + wa vectors ----
            abc = const.tile([128, 2 * FOUT], DT, name="abc", tag="abc")
            nc.gpsimd.partition_broadcast(abc[:], arow[0:1, :])
            wa1 = [const.tile([128, 1], DT, name=f"wa1_{i}", tag=f"wa1_{i}") for i in range(2)]
            # Wab_i = [W_i | W_i@a2 | 0.2*W_i@a2]  (single mm1 rhs; one
            # accumulation group per PSUM bank -- start=True clears the bank)
            wab = [const.tile([128, FOUT + 2], DT, name=f"wab{i}", tag=f"wab{i}") for i in range(2)]
            for i in range(2):
                t1 = wsm.tile([128, FOUT], DT, name="wtmp", tag="wtmp")
                nc.vector.tensor_tensor(t1[:], Wsb[i][:], abc[:, 0:FOUT], op=ALU.mult)
                nc.vector.reduce_sum(wa1[i][:, 0:1], t1[:], axis=mybir.AxisListType.X)
                t2 = wsm.tile([128, FOUT], DT, name="wtmp", tag="wtmp")
                nc.vector.tensor_tensor(t2[:], Wsb[i][:], abc[:, FOUT:2 * FOUT], op=ALU.mult)
                nc.vector.tensor_copy(wab[i][:, 0:FOUT], Wsb[i][:])
                nc.vector.reduce_sum(wab[i][:, FOUT:FOUT + 1], t2[:], axis=mybir.AxisListType.X)
                nc.vector.tensor_scalar_mul(wab[i][:, FOUT + 1:FOUT + 2], wab[i][:, FOUT:FOUT + 1], ALPHA)

            # ---- Wh1 row = a1^T W^T hT  -> broadcast to all partitions ----
            w1ps = pbig.tile([1, N], DT, name="big", tag="big")
            for ch in range(4):
                for i in range(2):
                    nc.tensor.matmul(
                        w1ps[0:1, ch * 512:(ch + 1) * 512],
                        wa1[i][:],
                        hT[i][:, ch * 512:(ch + 1) * 512],
                        start=(i == 0),
                        stop=(i == 1),
                    )
            w1row = const.tile([1, N], DT, name="w1row", tag="w1row")
            nc.vector.tensor_copy(w1row[:], w1ps[:])
            wh1b = const.tile([128, N], DT, name="wh1b", tag="wh1b")
            nc.gpsimd.partition_broadcast(wh1b[:], w1row[0:1, :])
            wh1b02 = const.tile([128, N], DT, name="wh1b02", tag="wh1b02")
            nc.vector.tensor_scalar_mul(wh1b02[:], wh1b[:], ALPHA)

            den = const.tile([128, NT], DT, name="den", tag="den")
            outp = pbig.tile([FOUT, N], DT, name="big", tag="big")

            rep_cms = (
                [tc.For_i(0, loop_k, 1)] if loop_k
                else [contextlib.nullcontext() for _ in range(reps)]
            )
            for rep_cm in rep_cms:
                ctx_val = rep_cm.__enter__()
                for mt in range(NT):
                    ms = mt * 128
                    # mm1: Wh tile [128m, 66] = [Wh | Wh@a2 | 0.2*Wh@a2]
                    whps = psmall.tile([128, FOUT + 2], DT, name="whps", tag="whps")
                    for i in range(2):
                        nc.tensor.matmul(
                            whps[:, 0:FOUT + 2],
                            hT[i][:, ms:ms + 128],
                            wab[i][:],
                            start=(i == 0),
                            stop=(i == 1),
                        )
                    wh2 = wsm.tile([128, 2], DT, name="wh2", tag="wh2")
                    nc.vector.tensor_copy(wh2[:], whps[:, FOUT:FOUT + 2])

                    # leaky(e) with e = Wh1[n] + Wh2[m]
                    att = watt.tile([128, N], DT, name="att", tag="att")
                    nc.scalar.activation(
                        att[:, 0:C_ACT], wh1b[:, 0:C_ACT], AF.Prelu,
                        bias=wh2[:, 0:1], scale=1.0, alpha=ALPHA,
                    )
                    ut = wut.tile([128, N - C_ACT], DT, name="ut", tag="ut")
                    nc.vector.tensor_scalar_add(ut[:], wh1b02[:, C_ACT:], wh2[:, 1:2])
                    nc.vector.scalar_tensor_tensor(
                        att[:, C_ACT:], wh1b[:, C_ACT:], wh2[:, 0:1], ut[:],
                        op0=ALU.add, op1=ALU.max,
                    )

                    # mask: att += {0, -57344} (fp8 -> f32 cast + add in DMA)
                    nc.gpsimd.dma_start(
                        att[:], adj8_d.ap()[ms:ms + 128, :], accum_op=ALU.add,
                    )

                    # P = exp(att), den = row-sum
                    ptile = wp.tile([128, N], DT, name="pt", tag="pt")
                    nc.scalar.activation(
                        ptile[:], att[:], AF.Exp, accum_out=den[:, mt:mt + 1],
                    )

                    # fold 1/den into Wh
                    rc = wsm.tile([128, 1], DT, name="rc", tag="rc")
                    nc.vector.reciprocal(rc[:], den[:, mt:mt + 1])
                    whp = wsm.tile([128, FOUT], DT, name="whp", tag="whp")
                    nc.vector.tensor_scalar_mul(whp[:], whps[:, 0:FOUT], rc[:, 0:1])

                    # mm2: out_T[o, n] += whp.T @ P
                    for ch in range(4):
                        nc.tensor.matmul(
                            outp[:, ch * 512:(ch + 1) * 512],
                            whp[:],
                            ptile[:, ch * 512:(ch + 1) * 512],
                            start=(mt == 0),
                            stop=(mt == NT - 1),
                        )

                # ---- ELU tail: elu(x) = relu(x) + min(exp(x)-1, 0) ----
                t_ = wout.tile([FOUT, N], DT, name="t", tag="t")
                r_ = wout.tile([FOUT, N], DT, name="r", tag="r")
                q_ = wout.tile([FOUT, N], DT, name="q", tag="q")
                osb = wout.tile([FOUT, N], DT, name="osb", tag="osb")
                nc.scalar.activation(t_[:], outp[:], AF.Exp)
                nc.scalar.activation(r_[:], outp[:], AF.Relu)
                nc.vector.tensor_scalar(
                    q_[:], t_[:], -1.0, 0.0, op0=ALU.add, op1=ALU.min,
                )
                nc.vector.tensor_tensor(osb[:], r_[:], q_[:], op=ALU.add)
                nc.sync.dma_start(out_d.ap(), osb[:])
                rep_cm.__exit__(None, None, None)

    nc.compile()
    return nc


def prepare_in_maps(h, adj, W, a):
    in_maps = []
    for b in range(B):
        hT = np.ascontiguousarray(h[b].T)
        adj8 = ((adj[b].T.astype(np.float32) - 1.0) * MASK8).astype(
            ml_dtypes.float8_e5m2
        )
        arow = np.ascontiguousarray(a[b].reshape(1, 2 * FOUT).astype(np.float32))
        in_maps.append(
            {
                "hT": hT,
                "W": np.ascontiguousarray(W[b]),
                "arow": arow,
                "adj8": adj8,
            }
        )
    return in_maps


def kernel(h, adj, W, a):
    """Full-input entry point: returns elu-GAT output [8, 2048, 64] float32."""
    if "nc" not in _CACHE:
        _CACHE["nc"] = build_program()
    nc = _CACHE["nc"]
    in_maps = prepare_in_maps(h, adj, W, a)
    res = bass_utils.run_bass_kernel_spmd(nc, in_maps, core_ids=list(range(B)))
    out = np.stack([res.results[b]["out"].T for b in range(B)])
    return np.ascontiguousarray(out.astype(np.float32))



# revision 18
# speedup vs baseline: 328.9212x; 328.9212x over previous
"""GraphAttentionLayer (GAT) Bass kernel for Trainium2, 8 NeuronCores.

Problem: B=8, N=2048, Fin=256, Fout=64
    Wh  = h @ W                                   [B, N, 64]
    e   = Wh@a1 + (Wh@a2)^T  (additive scores)    [B, N, N]
    att = where(adj>0, leaky_relu(e, 0.2), -9e15)
    A   = softmax(att, axis=1)   (column softmax!)
    out = elu(A @ Wh)

Sharding: batch-parallel, one graph per core (no communication).

Key algebra (per core; m = attended-over node on partitions, n = output
node along the free axis; e[n,m] = Wh1[n] + Wh2[m] is rank-1):

    exp(leaky(e)) = max(exp(e), exp(0.2 e))           (exp monotone)
    exp(e - C[m])     = E1[n] * 1        with C[m] = Wh2[m] + M
    exp(0.2e - C[m])  = E2[n] * F2[m]
      E1[n] = exp(Wh1[n] - M),  E2[n] = exp(0.2 Wh1[n]),
      F2[m] = exp(-0.8 Wh2[m] - M),  M = max(max Wh1, max -Wh2)

The per-column (per-m) shift C[m] cancels in the softmax and keeps every
unnormalized weight in (0, 1] -> the whole N^2 pipeline runs in fp16.

Per m-tile of 128 (the measured loop):
    mm1 (PE, f16): Wh[m,0:64] psum
    ACT route (cols 0:XA):  lx = Prelu(Wh1[n] + Wh2[m]);
                            t[:, 0:XA] = Exp(lx - C[m])
    DVE route (cols XA:N):  t[:, XA:] = max(E2b * F2[m], E1b)   (one stt)
    mask+den (DVE): P = min(t, adjT2), accum_out -> den[m]
        adjT2 = 2*adj^T in fp16 {0,2}: edge keeps t (t<=1), non-edge -> 0
    fold (DVE): whp = Wh[m,:] * (1/den[m])  -> f16
    mm2 (PE, f16): out_T[o,n] += whp.T @ P  (accumulate 16 m-tiles)
    elu tail: elu(x) = min(exp(min(x,11)) - 1, relu(x))
Host: transposes h/adj per batch (h/adj as fp16), transposes output back.
"""

import contextlib
import sys

import numpy as np

if "/opt/trn_rl_repo" not in sys.path:
    sys.path.append("/opt/trn_rl_repo")

import os

import ml_dtypes

import concourse.bass as bass
import concourse.bacc as bacc
import concourse.mybir as mybir
import concourse.tile as tile
from concourse import bass_utils

B = 8
N = 2048
FIN = 256
FOUT = 64
NT = N // 128          # 16 m-tiles
ALPHA = 0.2
XA = 992               # columns on the ACT (Prelu+Exp) route; rest on DVE
# Default (graded) configuration -- flip these to promote a variant.
DEFAULT_VARIANT = "full"
DEFAULT_XA = 1280
XA5 = 640              # v5's column split (baked into adj encoding)

DT = mybir.dt.float32
HALF = os.environ.get("GAT_HALF", "fp16")
F16 = mybir.dt.float16 if HALF == "fp16" else mybir.dt.bfloat16
NPH = np.float16 if HALF == "fp16" else ml_dtypes.bfloat16
AF = mybir.ActivationFunctionType
ALU = mybir.AluOpType

_CACHE = {}


def build_program(reps: int = 1, loop_k: int = 0, variant: str = "full", xa: int = XA,
                  gs: int = 0, stag: bool = False):
    """Build and compile the SPMD single-core program (identical on 8 cores).

    reps statically unrolls the main body; loop_k wraps it in a dynamic
    For_i loop instead (constant program size -- used for timing).
    variant: "full" | "dma" (adj DMAs only) | "nodma" (compute only,
    constant mask) | "nomm2" (no mm2/tail) -- non-"full" are timing-only.
    """
    nc = bacc.Bacc(
        "TRN2",
        target_bir_lowering=False,
        debug=False,
        enable_asserts=False,
        num_devices=B,
    )
    hT_d = nc.dram_tensor("hT16", [FIN, N], F16, kind="ExternalInput")
    W_d = nc.dram_tensor("W", [FIN, FOUT], DT, kind="ExternalInput")
    arow_d = nc.dram_tensor("arow", [1, 2 * FOUT], DT, kind="ExternalInput")
    adj_d = nc.dram_tensor("adjT2", [N, N], F16, kind="ExternalInput")
    out_d = nc.dram_tensor("out", [FOUT, N], F16, kind="ExternalOutput")

    with tile.TileContext(nc) as tc:
        with (
            tc.tile_pool(name="const", bufs=1) as const,
            tc.tile_pool(name="psmall", bufs=3, space=bass.MemorySpace.PSUM) as psmall,
            tc.tile_pool(name="pbig", bufs=1, space=bass.MemorySpace.PSUM) as pbig,
            tc.tile_pool(name="watt", bufs=3) as watt,
            tc.tile_pool(name="wt", bufs=3) as wt,
            tc.tile_pool(name="wlx", bufs=3) as wlx,
            tc.tile_pool(name="wp", bufs=3) as wp,
            tc.tile_pool(name="wsm", bufs=4) as wsm,
            tc.tile_pool(name="wout", bufs=1) as wout,
        ):
            # ---- load inputs ----
            hT = [const.tile([128, N], F16, name=f"hT{i}", tag=f"hT{i}") for i in range(2)]
            Wsb = [const.tile([128, FOUT], DT, name=f"W{i}", tag=f"W{i}") for i in range(2)]
            arow = const.tile([1, 2 * FOUT], DT, name="arow", tag="arow")
            for i in range(2):
                nc.sync.dma_start(hT[i][:], hT_d.ap()[i * 128:(i + 1) * 128, :])
                nc.sync.dma_start(Wsb[i][:], W_d.ap()[i * 128:(i + 1) * 128, :])
            nc.sync.dma_start(arow[:], arow_d.ap())

            # ---- W in f16 (mm1 rhs) ----
            W16 = [const.tile([128, FOUT], F16, name=f"W16_{i}", tag=f"W16_{i}") for i in range(2)]
            for i in range(2):
                nc.vector.tensor_copy(W16[i][:], Wsb[i][:])

            # ---- a broadcast + wa vectors (f32 math, f16 copies for PE) ----
            abc = const.tile([128, 2 * FOUT], DT, name="abc", tag="abc")
            nc.gpsimd.partition_broadcast(abc[:], arow[0:1, :])
            wa1_16 = [const.tile([128, 1], F16, name=f"wa1_{i}", tag=f"wa1_{i}") for i in range(2)]
            wa2_16 = [const.tile([128, 1], F16, name=f"wa2_{i}", tag=f"wa2_{i}") for i in range(2)]
            for i in range(2):
                t1 = wsm.tile([128, FOUT], DT, name="wtmp", tag="wtmp")
                nc.vector.tensor_tensor(t1[:], Wsb[i][:], abc[:, 0:FOUT], op=ALU.mult)
                s1 = wsm.tile([128, 1], DT, name="wsc", tag="wsc")
                nc.vector.reduce_sum(s1[:], t1[:], axis=mybir.AxisListType.X)
                nc.vector.tensor_copy(wa1_16[i][:], s1[:])
                t2 = wsm.tile([128, FOUT], DT, name="wtmp", tag="wtmp")
                nc.vector.tensor_tensor(t2[:], Wsb[i][:], abc[:, FOUT:2 * FOUT], op=ALU.mult)
                s2 = wsm.tile([128, 1], DT, name="wsc", tag="wsc")
                nc.vector.reduce_sum(s2[:], t2[:], axis=mybir.AxisListType.X)
                nc.vector.tensor_copy(wa2_16[i][:], s2[:])

            # ---- Wh1 / Wh2 rows over all n (PE) ----
            w1ps = pbig.tile([1, N], DT, name="big", tag="big")
            for ch in range(4):
                for i in range(2):
                    nc.tensor.matmul(
                        w1ps[0:1, ch * 512:(ch + 1) * 512],
                        wa1_16[i][:],
                        hT[i][:, ch * 512:(ch + 1) * 512],
                        start=(i == 0),
                        stop=(i == 1),
                    )
            w1row = const.tile([1, N], DT, name="w1row", tag="w1row")
            nc.vector.tensor_copy(w1row[:], w1ps[:])
            w2ps = pbig.tile([1, N], DT, name="big", tag="big")
            for ch in range(4):
                for i in range(2):
                    nc.tensor.matmul(
                        w2ps[0:1, ch * 512:(ch + 1) * 512],
                        wa2_16[i][:],
                        hT[i][:, ch * 512:(ch + 1) * 512],
                        start=(i == 0),
                        stop=(i == 1),
                    )
            w2row = const.tile([1, N], DT, name="w2row", tag="w2row")
            nc.vector.tensor_copy(w2row[:], w2ps[:])

            # ---- M = max(max Wh1, max -Wh2); negM = -M ----
            mx1 = wsm.tile([1, 1], DT, name="mx", tag="mx")
            nc.vector.reduce_max(mx1[:], w1row[:], axis=mybir.AxisListType.X)
            nw2 = wsm.tile([1, N], DT, name="nw2", tag="nw2")
            nc.vector.tensor_scalar_mul(nw2[:], w2row[:], -1.0)
            mx2 = wsm.tile([1, 1], DT, name="mx", tag="mx")
            nc.vector.reduce_max(mx2[:], nw2[:], axis=mybir.AxisListType.X)
            mxx = wsm.tile([1, 1], DT, name="mx", tag="mx")
            nc.vector.tensor_tensor(mxx[:], mx1[:], mx2[:], op=ALU.max)
            negM = const.tile([1, 1], DT, name="negM", tag="negM")
            nc.vector.tensor_scalar_mul(negM[:], mxx[:], -1.0)

            # ---- E rows (f16) + broadcasts ----
            e1row = const.tile([1, N], F16, name="e1row", tag="e1row")
            nc.scalar.activation(e1row[:], w1row[:], AF.Exp, bias=negM[0:1, 0:1], scale=1.0)
            e2row = const.tile([1, N], F16, name="e2row", tag="e2row")
            nc.scalar.activation(e2row[:], w1row[:], AF.Exp, scale=0.2)
            wh1b = const.tile([128, N], DT, name="wh1b", tag="wh1b")
            nc.gpsimd.partition_broadcast(wh1b[:], w1row[0:1, :])
            E1b = const.tile([128, N], F16, name="E1b", tag="E1b")
            nc.gpsimd.partition_broadcast(E1b[:], e1row[0:1, :])
            E2b = const.tile([128, N], F16, name="E2b", tag="E2b")
            nc.gpsimd.partition_broadcast(E2b[:], e2row[0:1, :])
            negMb = const.tile([128, 1], DT, name="negMb", tag="negMb")
            nc.gpsimd.partition_broadcast(negMb[:], negM[0:1, :])

            # ---- Wh2 per-partition columns (PE into psmall ring) ----
            whA = const.tile([128, NT], DT, name="whA", tag="whA")
            for mt in range(NT):
                ms = mt * 128
                wcps = psmall.tile([128, FOUT], DT, name="whps", tag="whps")
                for i in range(2):
                    nc.tensor.matmul(
                        wcps[:, 0:1],
                        hT[i][:, ms:ms + 128],
                        wa2_16[i][:],
                        start=(i == 0),
                        stop=(i == 1),
                    )
                nc.vector.tensor_copy(whA[:, mt:mt + 1], wcps[:, 0:1])
            # biasA = -(Wh2 + M);  F2all = exp(-0.8*Wh2 - M)
            biasA = const.tile([128, NT], DT, name="biasA", tag="biasA")
            nc.vector.tensor_scalar(
                biasA[:], whA[:], -1.0, negMb[:, 0:1], op0=ALU.mult, op1=ALU.add,
            )
            F2all = const.tile([128, NT], DT, name="F2all", tag="F2all")
            nc.scalar.activation(F2all[:], whA[:], AF.Exp, bias=negMb[:, 0:1], scale=-0.8)
            # biasB = -(0.8*Wh2 + M): ACT-b route exp(0.2*Wh1 + biasB) = exp(0.2e - C)
            biasB = const.tile([128, NT], DT, name="biasB", tag="biasB")
            nc.vector.tensor_scalar(
                biasB[:], whA[:], -0.8, negMb[:, 0:1], op0=ALU.mult, op1=ALU.add,
            )

            den = const.tile([128, NT], DT, name="den", tag="den")
            den2 = const.tile([128, 2 * NT], DT, name="den2", tag="den2")
            outp = pbig.tile([FOUT, N], DT, name="big", tag="big")
            cmask = None
            if variant == "nodma":
                cmask = const.tile([128, N], F16, name="cmask", tag="cmask")
                nc.vector.memset(cmask[:], 2.0)

            rep_cms = (
                [tc.For_i(0, loop_k, 1, staggered_reset=stag)] if loop_k
                else [contextlib.nullcontext() for _ in range(reps)]
            )
            for rep_cm in rep_cms:
                rep_cm.__enter__()
                if variant == "empty":
                    sink = wsm.tile([128, 16], F16, name="sink", tag="sink")
                    nc.vector.memset(sink[:], 1.0)
                    rep_cm.__exit__(None, None, None)
                    continue
                for mt in range(NT):
                    ms = mt * 128
                    # mask tile: adjT2[m, n] in {0, 2}
                    if variant == "nodma":
                        adjt = cmask
                    else:
                        adjt = watt.tile([128, N], F16, name="adjt", tag="adjt")
                        nc.sync.dma_start(adjt[:], adj_d.ap()[ms:ms + 128, :])
                    if variant == "dma":
                        sink = wsm.tile([128, 16], F16, name="sink", tag="sink")
                        nc.vector.tensor_copy(sink[:], adjt[:, 0:16])
                        continue

                    # mm1: Wh tile [128m, 64] (f16 inputs, f32 psum)
                    whps = psmall.tile([128, FOUT], DT, name="whps", tag="whps")
                    for i in range(2):
                        nc.tensor.matmul(
                            whps[:],
                            hT[i][:, ms:ms + 128],
                            W16[i][:],
                            start=(i == 0),
                            stop=(i == 1),
                        )

                    tfull = wt.tile([128, N], F16, name="tf", tag="tf")
                    if xa > 0:
                        # ACT route: lx = Prelu(Wh1[n] + Wh2[m]); t = Exp(lx - C[m])
                        lx = wlx.tile([128, xa], F16, name="lx", tag="lx")
                        nc.scalar.activation(
                            lx[:], wh1b[:, 0:xa], AF.Prelu,
                            bias=whA[:, mt:mt + 1], scale=1.0, alpha=ALPHA,
                        )
                        if variant != "v5":
                            nc.scalar.activation(
                                tfull[:, 0:xa], lx[:], AF.Exp,
                                bias=biasA[:, mt:mt + 1], scale=1.0,
                            )
                    ptile = wp.tile([128, N], F16, name="pt", tag="pt")
                    if variant == "v5":
                        # X-route [0:xa]: Prelu -> +adjneg (DVE) -> Exp+accum (ACT)
                        # Y-route [xa:]: bexp (ACT) -> max E1b -> min adj2 -> ts accum
                        lm = wlx.tile([128, xa], F16, name="lm", tag="lm")
                        nc.vector.tensor_tensor(
                            lm[:], lx[:], adjt[:, 0:xa], op=ALU.add,
                        )
                        nc.scalar.activation(
                            ptile[:, 0:xa], lm[:], AF.Exp,
                            bias=biasA[:, mt:mt + 1], scale=1.0,
                            accum_out=den2[:, 2 * mt:2 * mt + 1],
                        )
                        bexp = wlx.tile([128, N - xa], F16, name="bx", tag="bx")
                        nc.scalar.activation(
                            bexp[:], wh1b[:, xa:], AF.Exp,
                            bias=biasB[:, mt:mt + 1], scale=ALPHA,
                        )
                        nc.vector.tensor_tensor(
                            tfull[:, xa:], bexp[:], E1b[:, xa:], op=ALU.max,
                        )
                        nc.vector.tensor_tensor(
                            ptile[:, xa:], tfull[:, xa:], adjt[:, xa:], op=ALU.min,
                        )
                        scr = wt.tile([128, N - xa], F16, name="scr", tag="scr")
                        nc.vector.tensor_scalar(
                            scr[:], ptile[:, xa:], 1.0, 0.0, op0=ALU.mult,
                            op1=ALU.add,
                            accum_out=den2[:, 2 * mt + 1:2 * mt + 2],
                        )
                        nc.vector.tensor_tensor(
                            den[:, mt:mt + 1], den2[:, 2 * mt:2 * mt + 1],
                            den2[:, 2 * mt + 1:2 * mt + 2], op=ALU.add,
                        )
                    elif variant == "v4":
                        # b-branch on ACT; max split GPS [xa:xa+gs] / DVE [xa+gs:]
                        bexp = wlx.tile([128, N - xa], F16, name="bx", tag="bx")
                        nc.scalar.activation(
                            bexp[:], wh1b[:, xa:], AF.Exp,
                            bias=biasB[:, mt:mt + 1], scale=ALPHA,
                        )
                        if gs > 0:
                            nc.gpsimd.tensor_tensor(
                                tfull[:, xa:xa + gs], bexp[:, 0:gs],
                                E1b[:, xa:xa + gs], op=ALU.max,
                            )
                        if xa + gs < N:
                            nc.vector.tensor_tensor(
                                tfull[:, xa + gs:], bexp[:, gs:],
                                E1b[:, xa + gs:], op=ALU.max,
                            )
                        nc.vector.scalar_tensor_tensor(
                            ptile[:], tfull[:], 1.0, adjt[:],
                            op0=ALU.mult, op1=ALU.min,
                            accum_out=den[:, mt:mt + 1],
                        )
                    elif variant in ("v3s", "v3gpss"):
                        # b-branch on ACT, plain max, stt mask (immediate scalar)
                        bexp = wlx.tile([128, N - xa], F16, name="bx", tag="bx")
                        nc.scalar.activation(
                            bexp[:], wh1b[:, xa:], AF.Exp,
                            bias=biasB[:, mt:mt + 1], scale=ALPHA,
                        )
                        eng = nc.gpsimd if variant == "v3gpss" else nc.vector
                        eng.tensor_tensor(
                            tfull[:, xa:], bexp[:], E1b[:, xa:], op=ALU.max,
                        )
                        nc.vector.scalar_tensor_tensor(
                            ptile[:], tfull[:], 1.0, adjt[:],
                            op0=ALU.mult, op1=ALU.min,
                            accum_out=den[:, mt:mt + 1],
                        )
                    elif variant in ("v3", "v3gps"):
                        # b-branch on ACT: exp(0.2*Wh1[n] + biasB[m]) = exp(0.2e - C)
                        bexp = wlx.tile([128, N - xa], F16, name="bx", tag="bx")
                        nc.scalar.activation(
                            bexp[:], wh1b[:, xa:], AF.Exp,
                            bias=biasB[:, mt:mt + 1], scale=ALPHA,
                        )
                        # a-branch max (plain tensor_tensor, separate out)
                        eng = nc.gpsimd if variant == "v3gps" else nc.vector
                        eng.tensor_tensor(
                            tfull[:, xa:], bexp[:], E1b[:, xa:], op=ALU.max,
                        )
                        # mask + den split: P = min(t, adjT2)
                        if xa > 0:
                            nc.vector.tensor_tensor_reduce(
                                ptile[:, 0:xa], tfull[:, 0:xa], adjt[:, 0:xa],
                                1.0, 0.0, op0=ALU.min, op1=ALU.add,
                                accum_out=den2[:, 2 * mt:2 * mt + 1],
                            )
                        nc.vector.tensor_tensor_reduce(
                            ptile[:, xa:], tfull[:, xa:], adjt[:, xa:],
                            1.0, 0.0, op0=ALU.min, op1=ALU.add,
                            accum_out=den2[:, 2 * mt + 1:2 * mt + 2],
                        )
                        if xa > 0:
                            nc.vector.tensor_tensor(
                                den[:, mt:mt + 1], den2[:, 2 * mt:2 * mt + 1],
                                den2[:, 2 * mt + 1:2 * mt + 2], op=ALU.add,
                            )
                        else:
                            nc.vector.tensor_copy(
                                den[:, mt:mt + 1], den2[:, 2 * mt + 1:2 * mt + 2],
                            )
                    else:
                        if xa < N:
                            # DVE route: t = max(E2b * F2[m], E1b)
                            f2sc = 1.0 if variant == "immf2" else F2all[:, mt:mt + 1]
                            nc.vector.scalar_tensor_tensor(
                                tfull[:, xa:], E2b[:, xa:], f2sc,
                                E1b[:, xa:], op0=ALU.mult, op1=ALU.max,
                            )
                        # mask + den: P = min(t, adjT2), den = row-sum
                        nc.vector.scalar_tensor_tensor(
                            ptile[:], tfull[:], 1.0, adjt[:],
                            op0=ALU.mult, op1=ALU.min,
                            accum_out=den[:, mt:mt + 1],
                        )

                    if variant == "nomm2":
                        continue
                    # fold 1/den into Wh
                    rc = wsm.tile([128, 1], DT, name="rc", tag="rc")
                    nc.vector.reciprocal(rc[:], den[:, mt:mt + 1])
                    whp = wsm.tile([128, FOUT], F16, name="whp", tag="whp")
                    nc.vector.tensor_scalar_mul(whp[:], whps[:], rc[:, 0:1])

                    # mm2: out_T[o, n] += whp.T @ P
                    for ch in range(4):
                        nc.tensor.matmul(
                            outp[:, ch * 512:(ch + 1) * 512],
                            whp[:],
                            ptile[:, ch * 512:(ch + 1) * 512],
                            start=(mt == 0),
                            stop=(mt == NT - 1),
                        )

                if variant in ("dma", "nomm2"):
                    rep_cm.__exit__(None, None, None)
                    continue
                # ---- ELU tail: elu(x) = min(exp(x) - 1, relu(x)) ----
                if variant in ("v4", "v5"):
                    # f16 exp, no clamp: overflow saturates to inf/max and the
                    # final min() then picks the relu branch, which is correct.
                    q_ = wout.tile([FOUT, N], F16, name="q", tag="q")
                    nc.scalar.activation(q_[:], outp[:], AF.Exp)
                else:
                    qin = wout.tile([FOUT, N], F16, name="qin", tag="qin")
                    nc.vector.tensor_scalar_min(qin[:], outp[:], 11.0)
                    q_ = wout.tile([FOUT, N], F16, name="q", tag="q")
                    nc.scalar.activation(q_[:], qin[:], AF.Exp)
                r_ = wout.tile([FOUT, N], F16, name="r", tag="r")
                nc.vector.tensor_scalar_max(r_[:], outp[:], 0.0)
                osb = wout.tile([FOUT, N], F16, name="osb", tag="osb")
                nc.vector.scalar_tensor_tensor(
                    osb[:], q_[:], -1.0, r_[:], op0=ALU.add, op1=ALU.min,
                )
                nc.sync.dma_start(out_d.ap(), osb[:])
                rep_cm.__exit__(None, None, None)

    nc.compile()
    return nc




V5 = os.environ.get("GAT_V5", "1" if DEFAULT_VARIANT == "v5" else "0") == "1"


def prepare_in_maps(h, adj, W, a):
    in_maps = []
    for b in range(B):
        hT16 = np.ascontiguousarray(h[b].T).astype(NPH)
        adjT = np.ascontiguousarray(adj[b].T).astype(np.float32)
        if V5:
            # cols [0:XA5): additive mask {0,-1000}; cols [XA5:): min-mask {0,2}
            adjT2 = np.empty_like(adjT)
            adjT2[:, :XA5] = (adjT[:, :XA5] - 1.0) * 1000.0
            adjT2[:, XA5:] = adjT[:, XA5:] * 2.0
            adjT2 = adjT2.astype(NPH)
        else:
            adjT2 = (adjT * 2.0).astype(NPH)
        arow = np.ascontiguousarray(a[b].reshape(1, 2 * FOUT).astype(np.float32))
        in_maps.append(
            {
                "hT16": hT16,
                "W": np.ascontiguousarray(W[b]).astype(np.float32),
                "arow": arow,
                "adjT2": adjT2,
            }
        )
    return in_maps


def kernel(h, adj, W, a):
    """Full-input entry point: returns elu-GAT output [8, 2048, 64] float32."""
    if "nc" not in _CACHE:
        _CACHE["nc"] = build_program(
            variant=DEFAULT_VARIANT,
            xa=XA5 if DEFAULT_VARIANT == "v5" else DEFAULT_XA,
        )
    nc = _CACHE["nc"]
    in_maps = prepare_in_maps(h, adj, W, a)
    res = bass_utils.run_bass_kernel_spmd(nc, in_maps, core_ids=list(range(B)))
    out = np.stack([res.results[b]["out"].T.astype(np.float32) for b in range(B)])
    return np.ascontiguousarray(out)
